# revision 24
# baseline (speedup 1.0000x reference)
"""Trainium2 Bass kernel for nn_DAGT (gnn_message_passing), 8 NeuronCores.

Sharding: edges sorted by dst and sharded 512/core, nodes 256/core.
Each core computes full attention for its own queries (all 8 heads);
k (bf16) and v (fp8e4) are AllGathered per layer as separate
collectives so k-dependent score matmuls start earlier.  exp writes
fp8e4 scores directly; the AV matmul runs fp8 DoubleRow (2 key tiles
per pass at 0.5 cycles/row).  Softmax normalization is deferred to a
batched reciprocal_approx_fast + head-pair broadcast matmul.  All
edge-layer weights are preloaded at kernel start.  Per-graph pooling is
computed locally per core and combined with a tiny [B,H] AllReduce.
"""

import sys

for _p in ("/opt/trn_rl_repo",):
    if _p not in sys.path:
        sys.path.insert(0, _p)

import numpy as np

import concourse.bass as bass
import concourse.mybir as mybir
import concourse.tile as tile
from concourse.bass_utils import run_bass_kernel_spmd
from concourse.masks import make_identity
from concourse.vector_clock import ScopedClock

NC = 8
N, E, B = 2048, 4096, 8
H, L, NH, HD = 512, 3, 8, 64
ATOM_DIM, BOND_DIM = 41, 10
EC = E // NC  # 512 edges per core
NCC = N // NC  # 256 nodes per core
ET = EC // 128  # 4 own edge tiles
NT = NCC // 128  # 2 own node tiles
HT = H // 128  # 4 hidden tiles
VW = 66  # per-head stride in Vaug tiles (64 v dims + ones col + pad)

F32 = mybir.dt.float32
BF16 = mybir.dt.bfloat16
F8 = mybir.dt.float8e4
AF = mybir.ActivationFunctionType
ALU = mybir.AluOpType
DR = mybir.MatmulPerfMode.DoubleRow


def _patch_tile_drain():
    """walrus in this container caps sync-waits at 1 per plain instruction;
    split the Tile tail-drain waits across multiple drain instructions."""

    def _drain_and_barrier_split(self, tick_clock, wait_clock):
        drain_inst = self.nc.sync.drain()
        wait_clock.add_sem_waits(
            drain_inst.ins, ScopedClock({None: tick_clock.global_clock})
        )
        si = drain_inst.ins.sync_info
        if si is not None and len(si.on_wait) > 1:
            extra = list(si.on_wait[1:])
            del si.on_wait[1:]
            for w in extra:
                d2 = self.nc.sync.drain()
                d2.ins.sync_info = mybir.SyncInfo(on_wait=[w], on_update=[])
        self.nc.all_engine_barrier()
        assert self.sems is not None
        popped = self.nc._tile_sem_poison_stack.pop()
        assert popped is self._sem_poison
        self.nc.clear_and_free_semaphores(list(self.sems.allocated().values()))
        self.nc.all_engine_barrier()

    tile.TileContext._drain_and_barrier = _drain_and_barrier_split


_patch_tile_drain()


def _split_multi_waits(nc):
    """This walrus accepts at most 1 sync-wait per plain instruction (2 for
    event-semaphore ops).  Hoist extra waits onto preceding same-engine NOPs."""
    for f in nc.m.functions:
        for bb in f.blocks:
            new_insts = []
            for inst in bb.instructions:
                si = getattr(inst, "sync_info", None)
                cap = 2 if "EventSemaphore" in type(inst).__name__ else 1
                if si is not None and len(si.on_wait) > cap:
                    extra = list(si.on_wait[cap:])
                    del si.on_wait[cap:]
                    for w in extra:
                        nop = mybir.InstNoOp(
                            name=f"I-{nc.next_id()}",
                            engine=inst.engine,
                            sync_info=mybir.SyncInfo(on_wait=[w], on_update=[]),
                            bass_nofuse=True,
                        )
                        new_insts.append(nop)
                new_insts.append(inst)
            bb.instructions[:] = new_insts


def _bf(a):
    import ml_dtypes

    return np.ascontiguousarray(np.asarray(a, np.float32)).astype(ml_dtypes.bfloat16)


def _f32(a):
    return np.ascontiguousarray(np.asarray(a, np.float32))


# ---------------------------------------------------------------------------
# device kernel builder
# ---------------------------------------------------------------------------


def build_nc(fast: bool):
    nc = bass.Bass()

    di = {}

    def inp(name, shape, dt):
        di[name] = nc.dram_tensor(name, list(shape), dt, kind="ExternalInput")
        return di[name]

    inp("WQT", (L, H, H), BF16)
    inp("WKT", (L, H, H), BF16)
    inp("WVT", (L, H, H), BF16)
    inp("WOTT", (L, H, H), BF16)
    inp("WUPT", (L, H, H), BF16)
    inp("BQ", (L, H, 1), F32)
    inp("BK", (L, H, 1), F32)
    inp("BCAST", (23, 128, H), BF16)
    inp("bondWT", (BOND_DIM, H), BF16)
    inp("bondB", (H, 1), F32)
    inp("WHT", (H, H), BF16)
    inp("atomWT", (ATOM_DIM, H), BF16)
    inp("W1T", (H, H), BF16)
    inp("W2T", (H, H), BF16)
    inp("AQT", (H, H), BF16)
    inp("AKT", (H, H), BF16)
    inp("AVT", (H, H), BF16)
    inp("BAQ", (H, 1), F32)
    inp("BAK", (H, 1), F32)
    inp("AOTT", (H, H), BF16)
    inp("GP1T", (H, H), BF16)
    inp("GP2T", (H, H), BF16)
    inp("PB", (NCC, B), BF16)
    inp("eaT", (BOND_DIM, EC), BF16)
    inp("xT", (ATOM_DIM, NCC), BF16)
    inp("dege2", (EC, 1), F32)
    inp("cntinv", (NCC, 1), F32)
    if fast:
        inp("Ablk", (ET, 128, 128), BF16)
        inp("MtB", (ET, 128, NCC), BF16)
    else:
        inp("Ablk", (ET, E // 128, 128, 128), BF16)
        inp("MtB", (E // 128, 128, NCC), BF16)

    out = nc.dram_tensor("out", [B, H], F32, kind="ExternalOutput")

    kb = [nc.dram_tensor(f"kb{t}", [H, EC], F8) for t in range(L)]
    kg = [
        nc.dram_tensor(f"kg{t}", [NC, H, EC], F8, addr_space="Shared")
        for t in range(L)
    ]
    vb = [nc.dram_tensor(f"vb{t}", [EC, H], F8) for t in range(L)]
    vg = [
        nc.dram_tensor(f"vg{t}", [NC, EC, H], F8, addr_space="Shared")
        for t in range(L)
    ]
    nkb = nc.dram_tensor("nkb", [H, NCC], F8)
    nkg = nc.dram_tensor("nkg", [NC, H, NCC], F8, addr_space="Shared")
    nvb = nc.dram_tensor("nvb", [NCC, H], F8)
    nvg = nc.dram_tensor("nvg", [NC, NCC, H], F8, addr_space="Shared")
    prd_in = nc.dram_tensor("prd_in", [B, H], F32)
    prd_out = nc.dram_tensor("prd_out", [B, H], F32, addr_space="Shared")
    if not fast:
        hb = [nc.dram_tensor(f"hb{t}", [EC, H], BF16) for t in range(L + 1)]
        hg = [
            nc.dram_tensor(f"hg{t}", [E, H], BF16, addr_space="Shared")
            for t in range(L + 1)
        ]

    rg = [list(range(NC))]

    with tile.TileContext(nc) as tc:
        with (
            tc.tile_pool(name="const", bufs=1) as constp,
            tc.tile_pool(name="wpool", bufs=1) as wpool,
            tc.tile_pool(name="bc", bufs=1) as bcp_pool,
            tc.tile_pool(name="state", bufs=1) as statep,
            tc.tile_pool(name="work", bufs=1) as workp,
            tc.tile_pool(name="kvsb", bufs=1) as kvsb,
            tc.tile_pool(name="stream", bufs=4) as streamp,
            tc.tile_pool(name="expp", bufs=16) as expp,
            tc.tile_pool(name="small", bufs=4) as smallp,
            tc.tile_pool(name="recp", bufs=2) as recp,
            tc.tile_pool(name="psA", bufs=2, space="PSUM") as psA,
            tc.tile_pool(name="psB", bufs=2, space="PSUM") as psB,
            tc.tile_pool(name="psOE", bufs=1, space="PSUM") as psOE,
            tc.tile_pool(name="psT", bufs=1, space="PSUM") as psT,
        ):
            ident = constp.tile([128, 128], BF16, tag="ident", name="ident")
            make_identity(nc, ident[:])
            identf = constp.tile([128, 128], F32, tag="identf", name="identf")
            make_identity(nc, identf[:])
            eps1 = constp.tile([128, 1], F32, tag="eps1", name="eps1")
            nc.vector.memset(eps1[:], 1e-5)
            eps4 = constp.tile([128, 1], F32, tag="eps4", name="eps4")
            nc.vector.memset(eps4[:], 4e-5)
            sel_lo = constp.tile([1, 128], F32, tag="sel_lo", name="sel_lo")
            nc.vector.memset(sel_lo[:], 0.0)
            nc.vector.memset(sel_lo[0:1, 0:HD], 1.0)
            sel_hi = constp.tile([1, 128], F32, tag="sel_hi", name="sel_hi")
            nc.vector.memset(sel_hi[:], 0.0)
            nc.vector.memset(sel_hi[0:1, HD:128], 1.0)

            # ----- preload every weight -------------------------------------
            # edge layers: per-layer tags; node/gp reuse layer tags (read-after)
            eW = {}

            def load_edge_weights(t):
                for mi, mname in enumerate(("WQT", "WKT", "WVT", "WOTT", "WUPT")):
                    for it in range(HT):
                        tl = wpool.tile(
                            [128, H], BF16, tag=f"w{t % 2}_{mi}_{it}",
                            name=f"w{t}_{mi}_{it}",
                        )
                        nc.sync.dma_start(
                            tl[:], di[mname][t, it * 128:(it + 1) * 128, :]
                        )
                        eW[(t, mname, it)] = tl
            bondWT_sb = constp.tile([BOND_DIM, H], BF16, tag="bondWT", name="bondWT")
            nc.sync.dma_start(bondWT_sb[:], di["bondWT"][:])
            WHT_sb = [
                wpool.tile([128, H], BF16, tag=f"wh_{it}", name=f"wh_{it}")
                for it in range(HT)
            ]
            for it in range(HT):
                nc.sync.dma_start(WHT_sb[it][:], di["WHT"][it * 128:(it + 1) * 128, :])
            eaT_sb = constp.tile([BOND_DIM, EC], BF16, tag="eaT", name="eaT")
            nc.sync.dma_start(eaT_sb[:], di["eaT"][:])
            bondB_sb = [
                smallp.tile([128, 1], F32, tag="bondB", name="bondB")
                for _ in range(HT)
            ]
            for jt in range(HT):
                nc.sync.dma_start(
                    bondB_sb[jt][:], di["bondB"][jt * 128:(jt + 1) * 128, :]
                )
            load_edge_weights(0)
            bqL = [[None] * HT for _ in range(L)]
            bkL = [[None] * HT for _ in range(L)]
            for t in range(L):
                for jt in range(HT):
                    sl = slice(jt * 128, (jt + 1) * 128)
                    b1 = smallp.tile([128, 1], F32, tag=f"bq{t}_{jt}", name=f"bq{t}_{jt}")
                    nc.sync.dma_start(b1[:], di["BQ"][t, sl, :])
                    bqL[t][jt] = b1
                    b2 = smallp.tile([128, 1], F32, tag=f"bk{t}_{jt}", name=f"bk{t}_{jt}")
                    nc.sync.dma_start(b2[:], di["BK"][t, sl, :])
                    bkL[t][jt] = b2
            dege2_sb = [
                smallp.tile([128, 1], F32, tag=f"dege2_{et}", name=f"dege2_{et}")
                for et in range(ET)
            ]
            for et in range(ET):
                nc.sync.dma_start(
                    dege2_sb[et][:], di["dege2"][et * 128:(et + 1) * 128, :]
                )
            atomWT_sb = constp.tile([ATOM_DIM, H], BF16, tag="atomWT", name="atomWT")
            nc.sync.dma_start(atomWT_sb[:], di["atomWT"][:])
            xT_sb = constp.tile([ATOM_DIM, NCC], BF16, tag="xT", name="xT")
            nc.sync.dma_start(xT_sb[:], di["xT"][:])

            def bcast(idx, tag):
                t = bcp_pool.tile([128, H], BF16, tag=tag)
                nc.sync.dma_start(t[:], di["BCAST"][idx])
                return t

            def ln_tile(x_f32, out_t, eps_t, p=128, g=None, b=None, gelu=False):
                stats = smallp.tile([128, 1, 6], F32, tag="lnstats", name="lnstats")
                mv = smallp.tile([128, 2], F32, tag="lnmv", name="lnmv")
                nc.vector.bn_stats(out=stats[:p, 0, :], in_=x_f32)
                nc.vector.bn_aggr(out=mv[:p], in_=stats[:p])
                rstd = smallp.tile([128, 1], F32, tag="lnrstd", name="lnrstd")
                nc.scalar.activation(
                    out=rstd[:p], in_=mv[:p, 1:2], func=AF.Sqrt,
                    bias=eps_t[:p], scale=1.0,
                )
                nc.vector.reciprocal(out=rstd[:p], in_=rstd[:p])
                if g is None and b is None and not gelu:
                    nc.vector.tensor_scalar(
                        out=out_t, in0=x_f32, scalar1=mv[:p, 0:1],
                        scalar2=rstd[:p], op0=ALU.subtract, op1=ALU.mult,
                    )
                else:
                    y = workp.tile([128, H], F32, tag="lny", name="lny")
                    nc.vector.tensor_scalar(
                        out=y[:p, :], in0=x_f32, scalar1=mv[:p, 0:1],
                        scalar2=rstd[:p], op0=ALU.subtract, op1=ALU.mult,
                    )
                    if g is not None:
                        nc.vector.tensor_tensor(
                            out=y[:p, :], in0=y[:p, :], in1=g[:p, :], op=ALU.mult
                        )
                    if b is not None:
                        nc.vector.tensor_tensor(
                            out=y[:p, :], in0=y[:p, :], in1=b[:p, :], op=ALU.add
                        )
                    if gelu:
                        nc.scalar.activation(out=out_t, in_=y[:p, :], func=AF.Gelu)
                    else:
                        nc.vector.tensor_copy(out=out_t, in_=y[:p, :])

            def ln_group(items, eps_t, g=None, b=None, gelu=False, p=128):
                # items: list of (x_f32_ap, out_ap); shared sqrt/recip batch
                ng = len(items)
                mvg = smallp.tile([128, 2, 4], F32, tag="lnmvg", name="lnmvg")
                for i, (x_f32, _o) in enumerate(items):
                    stats = smallp.tile([128, 1, 6], F32, tag="lnstats", name="lnstats")
                    nc.vector.bn_stats(out=stats[:p, 0, :], in_=x_f32)
                    nc.vector.bn_aggr(out=mvg[:p, :, i], in_=stats[:p])
                rstd = smallp.tile([128, 4], F32, tag="lnrstdg", name="lnrstdg")
                nc.scalar.activation(
                    out=rstd[:p, :ng], in_=mvg[:p, 1, :ng], func=AF.Sqrt,
                    bias=eps_t[:p], scale=1.0,
                )
                nc.vector.reciprocal(out=rstd[:p, :ng], in_=rstd[:p, :ng])
                for i, (x_f32, out_t) in enumerate(items):
                    if g is None and b is None and not gelu:
                        nc.vector.tensor_scalar(
                            out=out_t, in0=x_f32, scalar1=mvg[:p, 0, i:i + 1],
                            scalar2=rstd[:p, i:i + 1], op0=ALU.subtract, op1=ALU.mult,
                        )
                    else:
                        y = workp.tile([128, H], F32, tag="lny", name="lny")
                        nc.vector.tensor_scalar(
                            out=y[:p, :], in0=x_f32, scalar1=mvg[:p, 0, i:i + 1],
                            scalar2=rstd[:p, i:i + 1], op0=ALU.subtract, op1=ALU.mult,
                        )
                        if g is not None:
                            nc.vector.tensor_tensor(
                                out=y[:p, :], in0=y[:p, :], in1=g[:p, :], op=ALU.mult
                            )
                        if b is not None:
                            nc.vector.tensor_tensor(
                                out=y[:p, :], in0=y[:p, :], in1=b[:p, :], op=ALU.add
                            )
                        if gelu:
                            nc.scalar.activation(out=out_t, in_=y[:p, :], func=AF.Gelu)
                        else:
                            nc.vector.tensor_copy(out=out_t, in_=y[:p, :])

            def transpose_128(src_ap, dst_ap, dtype_in):
                pt = psT.tile([128, 512], dtype_in, tag="trans", name="trans")
                idt = identf if dtype_in == F32 else ident
                nc.tensor.transpose(pt[:, :128], src_ap, idt[:])
                nc.vector.tensor_copy(out=dst_ap, in_=pt[:, :128])

            # ---------------- stage 0: bond embedding -------------------
            whb = bcast(0, "b5")

            tgT = [
                workp.tile([128, EC], BF16, tag=f"rlnT{jt}", name=f"rlnT{jt}")
                for jt in range(HT)
            ]
            for jt in range(HT):
                pt = psB.tile([128, H], F32, tag="ps_main", name="ps_main")
                nc.tensor.matmul(
                    pt[:, :EC],
                    bondWT_sb[:, jt * 128:(jt + 1) * 128],
                    eaT_sb[:],
                    start=True, stop=True,
                )
                nc.scalar.activation(
                    out=tgT[jt][:], in_=pt[:, :EC], func=AF.Gelu,
                    bias=bondB_sb[jt][:], scale=1.0,
                )

            h_own = [
                statep.tile([128, H], BF16, tag=f"hown{et}", name=f"hown{et}")
                for et in range(ET)
            ]
            for et in range(ET):
                pt = psB.tile([128, H], F32, tag="ps_main", name="ps_main")
                for it in range(HT):
                    nc.tensor.matmul(
                        pt[:],
                        tgT[it][:, et * 128:(et + 1) * 128],
                        WHT_sb[it][:],
                        start=(it == 0), stop=(it == HT - 1),
                    )
                nc.vector.tensor_tensor(
                    out=h_own[et][:], in0=pt[:], in1=whb[:], op=ALU.add
                )

            if fast:
                ab_sb = [
                    constp.tile([128, 128], BF16, tag=f"ab{et}", name=f"ab{et}")
                    for et in range(ET)
                ]
                for et in range(ET):
                    nc.sync.dma_start(ab_sb[et][:], di["Ablk"][et])
            else:
                for et in range(ET):
                    nc.sync.dma_start(hb[0][et * 128:(et + 1) * 128, :], h_own[et][:])
                nc.gpsimd.collective_compute(
                    "AllGather", ALU.bypass, replica_groups=rg,
                    ins=[hb[0][:]], outs=[hg[0][:]],
                )

            # atom embedding (independent of edge layers) — do it early
            atomb = bcast(13, "b5")
            atomg = bcast(14, "b2")
            atombb = bcast(15, "b3")
            a_i = [
                workp.tile([128, H], BF16, tag=f"ai{vt}", name=f"ai{vt}")
                for vt in range(NT)
            ]
            ab2s = []
            for vt in range(NT):
                pt = psB.tile([128, H], F32, tag="ps_main", name="ps_main")
                nc.tensor.matmul(
                    pt[:],
                    xT_sb[:, vt * 128:(vt + 1) * 128],
                    atomWT_sb[:],
                    start=True, stop=True,
                )
                ab2 = workp.tile([128, H], F32, tag=f"ub{vt}", name=f"ab2{vt}")
                nc.vector.tensor_tensor(out=ab2[:], in0=pt[:], in1=atomb[:], op=ALU.add)
                ab2s.append(ab2)
            ln_group(
                [(ab2s[vt][:], a_i[vt][:]) for vt in range(NT)],
                eps1, g=atomg, b=atombb, gelu=True,
            )
            aiT = [
                workp.tile([128, NCC], BF16, tag=f"aiT{it}", name=f"aiT{it}")
                for it in range(HT)
            ]
            for vt in range(NT):
                for it in range(HT):
                    transpose_128(
                        a_i[vt][:, it * 128:(it + 1) * 128],
                        aiT[it][:, vt * 128:(vt + 1) * 128],
                        BF16,
                    )

            # Vaug tiles (persistent; ones columns memset once)
            NKT = E // 128
            Vaug = [
                kvsb.tile([128, 2, NH * VW], F8, tag=f"Va{p}", name=f"Va{p}")
                for p in range(NKT // 2)
            ]
            for p in range(NKT // 2):
                va3 = Vaug[p].rearrange("p two (h w) -> p two h w", h=NH)
                for i in range(2):
                    nc.vector.memset(va3[:, i, :, HD:HD + 1], 1.0)
            NKT2 = N // 128
            nVaug = [
                kvsb.tile([128, 2, NH * VW], F8, tag=f"nVa{p}", name=f"nVa{p}")
                for p in range(NKT2 // 2)
            ]
            for p in range(NKT2 // 2):
                va3 = nVaug[p].rearrange("p two (h w) -> p two h w", h=NH)
                for i in range(2):
                    nc.vector.memset(va3[:, i, :, HD:HD + 1], 1.0)

            # ---------------- edge transformer layers -------------------
            for t in range(L):
                if t == 0:
                    load_edge_weights(1)
                if t == 1:
                    load_edge_weights(2)
                WQT_sb = [eW[(t, "WQT", it)] for it in range(HT)]
                WKT_sb = [eW[(t, "WKT", it)] for it in range(HT)]
                WVT_sb = [eW[(t, "WVT", it)] for it in range(HT)]
                WOTT_sb = [eW[(t, "WOTT", it)] for it in range(HT)]
                WUPT_sb = [eW[(t, "WUPT", it)] for it in range(HT)]
                bq_sb = bqL[t]
                bk_sb = bkL[t]
                bv = bcast(1 + t, "b0")
                updb = bcast(4 + t, "b1")
                updg = bcast(7 + t, "b2")
                updbb = bcast(10 + t, "b3")

                # r2 = 2*(S[dst] - deg*h) for own rows
                r2 = [
                    statep.tile([128, H], F32, tag=f"r2_{et}", name=f"r2_{et}")
                    for et in range(ET)
                ]
                for et in range(ET):
                    pr = psB.tile([128, H], F32, tag="ps_main", name="ps_main")
                    if fast:
                        nc.tensor.matmul(
                            pr[:], ab_sb[et][:], h_own[et][:], start=True, stop=True
                        )
                    else:
                        nj = E // 128
                        for jt in range(nj):
                            hj = streamp.tile([128, H], BF16, tag="hfull", name="hfull")
                            nc.sync.dma_start(
                                hj[:], hg[t][jt * 128:(jt + 1) * 128, :]
                            )
                            abj = streamp.tile([128, 128], BF16, tag="abj", name="abj")
                            nc.sync.dma_start(abj[:], di["Ablk"][et, jt])
                            nc.tensor.matmul(
                                pr[:], abj[:], hj[:],
                                start=(jt == 0), stop=(jt == nj - 1),
                            )
                    tmp = workp.tile([128, H], F32, tag="rtmp", name="rtmp")
                    nc.vector.tensor_scalar_mul(
                        out=tmp[:], in0=h_own[et][:], scalar1=dege2_sb[et][:]
                    )
                    nc.vector.tensor_tensor(
                        out=r2[et][:], in0=pr[:], in1=tmp[:], op=ALU.subtract
                    )

                rln = [
                    workp.tile([128, H], BF16, tag=f"rln{et}", name=f"rln{et}")
                    for et in range(ET)
                ]
                ln_group([(r2[et][:], rln[et][:]) for et in range(ET)], eps4)

                rlnT = [
                    workp.tile([128, EC], BF16, tag=f"rlnT{it}", name=f"rlnT{it}")
                    for it in range(HT)
                ]
                for et in range(ET):
                    for it in range(HT):
                        transpose_128(
                            rln[et][:, it * 128:(it + 1) * 128],
                            rlnT[it][:, et * 128:(et + 1) * 128],
                            BF16,
                        )

                # k first (feeds collective), then v, then q
                kT_own = [
                    workp.tile([128, EC], F8, tag=f"kTo{jt}", name=f"kTo{jt}")
                    for jt in range(HT)
                ]
                for jt in range(HT):
                    pk = psB.tile([128, H], F32, tag="ps_main", name="ps_main")
                    for it in range(HT):
                        nc.tensor.matmul(
                            pk[:, :EC],
                            WKT_sb[it][:, jt * 128:(jt + 1) * 128],
                            rlnT[it][:],
                            start=(it == 0), stop=(it == HT - 1),
                        )
                    nc.vector.tensor_scalar_add(
                        out=kT_own[jt][:], in0=pk[:, :EC], scalar1=bk_sb[jt][:]
                    )
                    nc.sync.dma_start(
                        kb[t][jt * 128:(jt + 1) * 128, :], kT_own[jt][:]
                    )
                nc.gpsimd.collective_compute(
                    "AllGather", ALU.bypass, replica_groups=rg,
                    ins=[kb[t][:]], outs=[kg[t][:]],
                )

                v8_own = [
                    workp.tile([128, H], F8, tag=f"v8o{et}", name=f"v8o{et}")
                    for et in range(ET)
                ]
                for et in range(ET):
                    pv = psB.tile([128, H], F32, tag="ps_main", name="ps_main")
                    for it in range(HT):
                        nc.tensor.matmul(
                            pv[:],
                            rlnT[it][:, et * 128:(et + 1) * 128],
                            WVT_sb[it][:],
                            start=(it == 0), stop=(it == HT - 1),
                        )
                    nc.vector.tensor_tensor(
                        out=v8_own[et][:], in0=pv[:], in1=bv[:], op=ALU.add
                    )
                    nc.sync.dma_start(
                        vb[t][et * 128:(et + 1) * 128, :], v8_own[et][:]
                    )
                nc.gpsimd.collective_compute(
                    "AllGather", ALU.bypass, replica_groups=rg,
                    ins=[vb[t][:]], outs=[vg[t][:]],
                )

                qT = [
                    workp.tile([128, EC], F8, tag=f"qT{jt}", name=f"qT{jt}")
                    for jt in range(HT)
                ]
                for jt in range(HT):
                    pq = psB.tile([128, H], F32, tag="ps_main", name="ps_main")
                    for it in range(HT):
                        nc.tensor.matmul(
                            pq[:, :EC],
                            WQT_sb[it][:, jt * 128:(jt + 1) * 128],
                            rlnT[it][:],
                            start=(it == 0), stop=(it == HT - 1),
                        )
                    nc.vector.tensor_scalar_add(
                        out=qT[jt][:], in0=pq[:, :EC], scalar1=bq_sb[jt][:]
                    )

                # readback: K jt-major so head 0 can start earliest
                KT = [
                    kvsb.tile([128, E], F8, tag=f"KT{jt}", name=f"KT{jt}")
                    for jt in range(HT)
                ]
                for jt in range(HT):
                    for cp in range(NC):
                        nc.sync.dma_start(
                            KT[jt][:, cp * EC:(cp + 1) * EC],
                            kg[t][cp, jt * 128:(jt + 1) * 128, :],
                        )
                for cp in range(NC):
                    for rt in range(ET):
                        kt = cp * ET + rt
                        p, i = kt // 2, kt % 2
                        va3 = Vaug[p].rearrange("p two (h w) -> p two h w", h=NH)
                        nc.gpsimd.dma_start(
                            va3[:, i, :, 0:HD],
                            vg[t][cp, rt * 128:(rt + 1) * 128, :].rearrange(
                                "p (h w) -> p h w", h=NH
                            ),
                        )

                # r2T transposes ride the collective/readback wait window
                r2T = [
                    workp.tile([128, EC], BF16, tag=f"r2T{it}", name=f"r2T{it}")
                    for it in range(HT)
                ]
                for et in range(ET):
                    for it in range(HT):
                        transpose_128(
                            r2[et][:, it * 128:(it + 1) * 128],
                            r2T[it][:, et * 128:(et + 1) * 128],
                            F32,
                        )

                # attention: per head, 16 kt-pairs: 2 score mm + exp + AV-DR
                oT = [
                    workp.tile([128, EC], BF16, tag=f"oT{it}", name=f"oT{it}")
                    for it in range(HT)
                ]
                oes_all = {}
                for h in range(NH):
                    jt, po = h // 2, (h % 2) * HD
                    q_h = qT[jt][po:po + HD, :]
                    oe = psOE.tile([128, H], F32, tag="ps_oext", name="ps_oext")
                    es_list = []
                    for bi in range(NKT // 2):
                        ps = psA.tile([128, 2, EC], F32, tag="ps_scores", name="ps_scores")
                        es = expp.tile([128, 2, EC], F8, tag="exps", name="exps")
                        for kk in range(2):
                            kt = bi * 2 + kk
                            nc.tensor.matmul(
                                ps[:, kk, :],
                                KT[jt][po:po + HD, kt * 128:(kt + 1) * 128],
                                q_h, start=True, stop=True,
                            )
                        nc.scalar.activation(
                            out=es[:].rearrange("p a e -> p (a e)"),
                            in_=ps[:].rearrange("p a e -> p (a e)"),
                            func=AF.Exp,
                        )
                        es_list.append(es)
                    for bi in range(NKT // 2):
                        nc.tensor.matmul(
                            oe[:HD + 1, :EC],
                            Vaug[bi][:, :, h * VW:h * VW + HD + 1],
                            es_list[bi][:],
                            start=(bi == 0), stop=(bi == NKT // 2 - 1),
                            perf_mode=DR,
                        )
                    # stash denominator reciprocal + numerator copy; free oe
                    if h % 2 == 0:
                        rec_pair = recp.tile([1, 2 * EC], F32, tag="rec", name="rec")
                    nc.vector.tensor_copy(
                        out=rec_pair[:, (h % 2) * EC:(h % 2 + 1) * EC],
                        in_=oe[HD:HD + 1, :EC],
                    )
                    oes = workp.tile([128, EC], BF16, tag=f"oes{h % 2}", name=f"oes{h % 2}")
                    nc.vector.tensor_copy(out=oes[:HD, :], in_=oe[:HD, :EC])
                    oes_all[h] = oes
                    if h % 2 == 1:
                        nc.vector.reciprocal(out=rec_pair[:], in_=rec_pair[:])
                        bcm = psT.tile([128, 512], F32, tag="trans", name="trans")
                        nc.tensor.matmul(
                            bcm[:, :EC], sel_lo[:], rec_pair[:, 0:EC],
                            start=True, stop=False,
                        )
                        nc.tensor.matmul(
                            bcm[:, :EC], sel_hi[:], rec_pair[:, EC:],
                            start=False, stop=True,
                        )
                        nc.vector.tensor_tensor(
                            out=oT[jt][0:HD, :], in0=oes_all[h - 1][:HD, :],
                            in1=bcm[0:HD, :EC], op=ALU.mult,
                        )
                        nc.vector.tensor_tensor(
                            out=oT[jt][HD:128, :], in0=oes_all[h][:HD, :],
                            in1=bcm[HD:128, :EC], op=ALU.mult,
                        )

                # out-proj (transposed) + residual -> tijT
                tijT = [
                    workp.tile([128, EC], BF16, tag=f"tijT{jt}", name=f"tijT{jt}")
                    for jt in range(HT)
                ]
                for jt in range(HT):
                    pa = psB.tile([128, H], F32, tag="ps_main", name="ps_main")
                    for it in range(HT):
                        nc.tensor.matmul(
                            pa[:, :EC],
                            WOTT_sb[it][:, jt * 128:(jt + 1) * 128],
                            oT[it][:],
                            start=(it == 0), stop=(it == HT - 1),
                        )
                    nc.vector.tensor_tensor(
                        out=tijT[jt][:], in0=pa[:, :EC], in1=r2T[jt][:], op=ALU.add
                    )

                # update + LN + GELU -> new h_own
                ubs = []
                for et in range(ET):
                    pu = psB.tile([128, H], F32, tag="ps_main", name="ps_main")
                    for jt in range(HT):
                        nc.tensor.matmul(
                            pu[:],
                            tijT[jt][:, et * 128:(et + 1) * 128],
                            WUPT_sb[jt][:],
                            start=(jt == 0), stop=(jt == HT - 1),
                        )
                    ub = workp.tile([128, H], F32, tag=f"ub{et}", name=f"ub{et}")
                    nc.vector.tensor_tensor(
                        out=ub[:], in0=pu[:], in1=updb[:], op=ALU.add
                    )
                    ubs.append(ub)
                    h_own[et] = statep.tile(
                        [128, H], BF16, tag=f"hown{et}", name=f"hown{et}"
                    )
                ln_group(
                    [(ubs[et][:], h_own[et][:]) for et in range(ET)],
                    eps1, g=updg, b=updbb, gelu=True,
                )

                if not fast:
                    for et in range(ET):
                        nc.sync.dma_start(
                            hb[t + 1][et * 128:(et + 1) * 128, :], h_own[et][:]
                        )
                    nc.gpsimd.collective_compute(
                        "AllGather", ALU.bypass, replica_groups=rg,
                        ins=[hb[t + 1][:]], outs=[hg[t + 1][:]],
                    )

            # ---------------- node phase --------------------------------
            W1T_sb = [eW[(1, "WQT", it)] for it in range(HT)]
            W2T_sb = [eW[(1, "WKT", it)] for it in range(HT)]
            AQT_sb = [eW[(1, "WVT", it)] for it in range(HT)]
            AKT_sb = [eW[(1, "WOTT", it)] for it in range(HT)]
            AVT_sb = [eW[(1, "WUPT", it)] for it in range(HT)]
            AOTT_sb = [eW[(2, "WQT", it)] for it in range(HT)]
            for it in range(HT):
                sl = slice(it * 128, (it + 1) * 128)
                nc.sync.dma_start(W1T_sb[it][:], di["W1T"][sl, :])
                nc.sync.dma_start(W2T_sb[it][:], di["W2T"][sl, :])
                nc.sync.dma_start(AQT_sb[it][:], di["AQT"][sl, :])
                nc.sync.dma_start(AKT_sb[it][:], di["AKT"][sl, :])
                nc.sync.dma_start(AVT_sb[it][:], di["AVT"][sl, :])
                nc.sync.dma_start(AOTT_sb[it][:], di["AOTT"][sl, :])
            baq_sb = [smallp.tile([128, 1], F32, tag="baq", name="baq") for _ in range(HT)]
            bak_sb = [smallp.tile([128, 1], F32, tag="bak", name="bak") for _ in range(HT)]
            for jt in range(HT):
                sl = slice(jt * 128, (jt + 1) * 128)
                nc.sync.dma_start(baq_sb[jt][:], di["BAQ"][sl, :])
                nc.sync.dma_start(bak_sb[jt][:], di["BAK"][sl, :])
            featb2 = bcast(16, "b1")
            bav = bcast(17, "b0")
            aob = bcast(18, "b4")

            # S2^T = (segment_sum of final h by dst, own nodes)^T
            s2T = [
                workp.tile([128, NCC], BF16, tag=f"rlnT{jt}", name=f"s2T{jt}")
                for jt in range(HT)
            ]
            n_eb = ET if fast else E // 128
            MtB_sb = []
            for eb in range(n_eb):
                mt = kvsb.tile([128, NCC], BF16, tag=f"mtb{eb}", name=f"mtb{eb}")
                nc.sync.dma_start(mt[:], di["MtB"][eb])
                MtB_sb.append(mt)
            if not fast:
                hfin = []
                for jt in range(E // 128):
                    hj = kvsb.tile([128, H], BF16, tag=f"hfin{jt}", name=f"hfin{jt}")
                    nc.sync.dma_start(hj[:], hg[L][jt * 128:(jt + 1) * 128, :])
                    hfin.append(hj)
            for jt in range(HT):
                pt = psB.tile([128, H], F32, tag="ps_main", name="ps_main")
                for eb in range(n_eb):
                    lhs = h_own[eb] if fast else hfin[eb]
                    nc.tensor.matmul(
                        pt[:, :NCC],
                        lhs[:, jt * 128:(jt + 1) * 128],
                        MtB_sb[eb][:],
                        start=(eb == 0), stop=(eb == n_eb - 1),
                    )
                nc.vector.tensor_copy(out=s2T[jt][:], in_=pt[:, :NCC])

            # x2 = 2*x_i
            x2 = [
                statep.tile([128, H], F32, tag=f"r2_{vt}", name=f"x2_{vt}")
                for vt in range(NT)
            ]
            for vt in range(NT):
                pt = psB.tile([128, H], F32, tag="ps_main", name="ps_main")
                for it in range(HT):
                    nc.tensor.matmul(
                        pt[:],
                        aiT[it][:, vt * 128:(vt + 1) * 128],
                        W1T_sb[it][:],
                        start=(it == 0), stop=False,
                    )
                for it in range(HT):
                    nc.tensor.matmul(
                        pt[:],
                        s2T[it][:, vt * 128:(vt + 1) * 128],
                        W2T_sb[it][:],
                        start=False, stop=(it == HT - 1),
                    )
                nc.vector.tensor_tensor(
                    out=x2[vt][:], in0=pt[:], in1=featb2[:], op=ALU.add
                )

            lnxi = [
                workp.tile([128, H], BF16, tag=f"rln{vt}", name=f"lnxi{vt}")
                for vt in range(NT)
            ]
            ln_group([(x2[vt][:], lnxi[vt][:]) for vt in range(NT)], eps4)
            lnxiT = [
                workp.tile([128, NCC], BF16, tag=f"aiT{it}", name=f"lnxiT{it}")
                for it in range(HT)
            ]
            for vt in range(NT):
                for it in range(HT):
                    transpose_128(
                        lnxi[vt][:, it * 128:(it + 1) * 128],
                        lnxiT[it][:, vt * 128:(vt + 1) * 128],
                        BF16,
                    )
            # node k first, then v (collectives), then q
            nkT = [
                workp.tile([128, NCC], F8, tag=f"kTo{jt}", name=f"nkT{jt}")
                for jt in range(HT)
            ]
            for jt in range(HT):
                pk = psB.tile([128, H], F32, tag="ps_main", name="ps_main")
                for it in range(HT):
                    nc.tensor.matmul(
                        pk[:, :NCC],
                        AKT_sb[it][:, jt * 128:(jt + 1) * 128],
                        lnxiT[it][:],
                        start=(it == 0), stop=(it == HT - 1),
                    )
                nc.vector.tensor_scalar_add(
                    out=nkT[jt][:], in0=pk[:, :NCC], scalar1=bak_sb[jt][:]
                )
                nc.sync.dma_start(nkb[jt * 128:(jt + 1) * 128, :], nkT[jt][:])
            nc.gpsimd.collective_compute(
                "AllGather", ALU.bypass, replica_groups=rg,
                ins=[nkb[:]], outs=[nkg[:]],
            )
            nv8 = [
                workp.tile([128, H], F8, tag=f"v8o{vt}", name=f"nv8{vt}")
                for vt in range(NT)
            ]
            for vt in range(NT):
                pv = psB.tile([128, H], F32, tag="ps_main", name="ps_main")
                for it in range(HT):
                    nc.tensor.matmul(
                        pv[:],
                        lnxiT[it][:, vt * 128:(vt + 1) * 128],
                        AVT_sb[it][:],
                        start=(it == 0), stop=(it == HT - 1),
                    )
                nc.vector.tensor_tensor(
                    out=nv8[vt][:], in0=pv[:], in1=bav[:], op=ALU.add
                )
                nc.sync.dma_start(nvb[vt * 128:(vt + 1) * 128, :], nv8[vt][:])
            nc.gpsimd.collective_compute(
                "AllGather", ALU.bypass, replica_groups=rg,
                ins=[nvb[:]], outs=[nvg[:]],
            )
            nqT = [
                workp.tile([128, NCC], F8, tag=f"qT{jt}", name=f"nqT{jt}")
                for jt in range(HT)
            ]
            for jt in range(HT):
                pq = psB.tile([128, H], F32, tag="ps_main", name="ps_main")
                for it in range(HT):
                    nc.tensor.matmul(
                        pq[:, :NCC],
                        AQT_sb[it][:, jt * 128:(jt + 1) * 128],
                        lnxiT[it][:],
                        start=(it == 0), stop=(it == HT - 1),
                    )
                nc.vector.tensor_scalar_add(
                    out=nqT[jt][:], in0=pq[:, :NCC], scalar1=baq_sb[jt][:]
                )

            nKT = [
                kvsb.tile([128, N], F8, tag=f"KT{jt}", name=f"nKT{jt}")
                for jt in range(HT)
            ]
            for jt in range(HT):
                for cp in range(NC):
                    nc.sync.dma_start(
                        nKT[jt][:, cp * NCC:(cp + 1) * NCC],
                        nkg[cp, jt * 128:(jt + 1) * 128, :],
                    )
            for cp in range(NC):
                for rt in range(NT):
                    kt = cp * NT + rt
                    p, i = kt // 2, kt % 2
                    va3 = nVaug[p].rearrange("p two (h w) -> p two h w", h=NH)
                    nc.gpsimd.dma_start(
                        va3[:, i, :, 0:HD],
                        nvg[cp, rt * 128:(rt + 1) * 128, :].rearrange(
                            "p (h w) -> p h w", h=NH
                        ),
                    )

            # node attention: 4 kts per exp op, AV-DR over 2 pairs per op
            noT = [
                workp.tile([128, NCC], BF16, tag=f"oT{it}", name=f"noT{it}")
                for it in range(HT)
            ]
            noes_all = {}
            for h in range(NH):
                jt, po = h // 2, (h % 2) * HD
                q_h = nqT[jt][po:po + HD, :]
                oe = psOE.tile([128, H], F32, tag="ps_oext", name="ps_oext")
                es_list = []
                for bi in range(NKT2 // 4):
                    ps = psA.tile([128, 4, NCC], F32, tag="ps_scores", name="ps_scores")
                    es = expp.tile([128, 4, NCC], F8, tag="exps", name="exps")
                    for kk in range(4):
                        kt = bi * 4 + kk
                        nc.tensor.matmul(
                            ps[:, kk, :],
                            nKT[jt][po:po + HD, kt * 128:(kt + 1) * 128],
                            q_h, start=True, stop=True,
                        )
                    nc.scalar.activation(
                        out=es[:].rearrange("p a e -> p (a e)"),
                        in_=ps[:].rearrange("p a e -> p (a e)"),
                        func=AF.Exp,
                    )
                    es_list.append(es)
                for bi in range(NKT2 // 4):
                    for pp in range(2):
                        p = bi * 2 + pp
                        nc.tensor.matmul(
                            oe[:HD + 1, :NCC],
                            nVaug[p][:, :, h * VW:h * VW + HD + 1],
                            es_list[bi][:, 2 * pp:2 * pp + 2, :],
                            start=(p == 0), stop=(p == NKT2 // 2 - 1),
                            perf_mode=DR,
                        )
                if h % 2 == 0:
                    rec_pair = recp.tile([1, 2 * EC], F32, tag="rec", name="nrec")
                nc.vector.tensor_copy(
                    out=rec_pair[:, (h % 2) * EC:(h % 2) * EC + NCC],
                    in_=oe[HD:HD + 1, :NCC],
                )
                oes = workp.tile([128, EC], BF16, tag=f"oes{h % 2}", name=f"noes{h % 2}")
                nc.vector.tensor_copy(out=oes[:HD, :NCC], in_=oe[:HD, :NCC])
                noes_all[h] = oes
                if h % 2 == 1:
                    nc.vector.reciprocal(out=rec_pair[:], in_=rec_pair[:])
                    bcm = psT.tile([128, 512], F32, tag="trans", name="trans")
                    nc.tensor.matmul(
                        bcm[:, :NCC], sel_lo[:], rec_pair[:, 0:NCC],
                        start=True, stop=False,
                    )
                    nc.tensor.matmul(
                        bcm[:, :NCC], sel_hi[:], rec_pair[:, EC:EC + NCC],
                        start=False, stop=True,
                    )
                    nc.vector.tensor_tensor(
                        out=noT[jt][0:HD, :], in0=noes_all[h - 1][:HD, :NCC],
                        in1=bcm[0:HD, :NCC], op=ALU.mult,
                    )
                    nc.vector.tensor_tensor(
                        out=noT[jt][HD:128, :], in0=noes_all[h][:HD, :NCC],
                        in1=bcm[HD:128, :NCC], op=ALU.mult,
                    )

            # h_node = (o @ ao^T + aob + x2) * cntinv ; local per-graph pool
            cntinv_sb = [
                smallp.tile([128, 1], F32, tag="cntinv", name="cntinv")
                for _ in range(NT)
            ]
            for vt in range(NT):
                nc.sync.dma_start(
                    cntinv_sb[vt][:], di["cntinv"][vt * 128:(vt + 1) * 128, :]
                )
            PB_sb = [
                smallp.tile([128, B], BF16, tag=f"pb{vt}", name=f"pb{vt}")
                for vt in range(NT)
            ]
            for vt in range(NT):
                nc.sync.dma_start(PB_sb[vt][:], di["PB"][vt * 128:(vt + 1) * 128, :])

            pg = psT.tile([128, 512], F32, tag="trans", name="pgsum")
            for vt in range(NT):
                pa = psB.tile([128, H], F32, tag="ps_main", name="ps_main")
                for it in range(HT):
                    nc.tensor.matmul(
                        pa[:],
                        noT[it][:, vt * 128:(vt + 1) * 128],
                        AOTT_sb[it][:],
                        start=(it == 0), stop=(it == HT - 1),
                    )
                hn = workp.tile([128, H], F32, tag="ub", name="ub")
                nc.vector.tensor_tensor(out=hn[:], in0=pa[:], in1=aob[:], op=ALU.add)
                nc.vector.tensor_tensor(out=hn[:], in0=hn[:], in1=x2[vt][:], op=ALU.add)
                hnb16 = workp.tile([128, H], BF16, tag="hnb16", name="hnb16")
                nc.vector.tensor_scalar_mul(
                    out=hnb16[:], in0=hn[:], scalar1=cntinv_sb[vt][:]
                )
                nc.tensor.matmul(
                    pg[:B, :], PB_sb[vt][:], hnb16[:],
                    start=(vt == 0), stop=(vt == NT - 1),
                )
            pgf = workp.tile([128, H], F32, tag="pgf", name="pgf")
            nc.vector.tensor_copy(out=pgf[:B, :], in_=pg[:B, :])
            nc.sync.dma_start(prd_in[:], pgf[:B, :])
            nc.gpsimd.collective_compute(
                "AllReduce", ALU.add, replica_groups=rg,
                ins=[prd_in[:]], outs=[prd_out[:]],
            )
            hgsum = workp.tile([128, H], F32, tag="pgf", name="hgsum")
            nc.sync.dma_start(hgsum[:B, :], prd_out[:])

            # graph head (redundant on every core)
            GP1T_sb = [eW[(2, "WKT", it)] for it in range(HT)]
            GP2T_sb = [eW[(2, "WVT", it)] for it in range(HT)]
            for it in range(HT):
                sl = slice(it * 128, (it + 1) * 128)
                nc.sync.dma_start(GP1T_sb[it][:], di["GP1T"][sl, :])
                nc.sync.dma_start(GP2T_sb[it][:], di["GP2T"][sl, :])
            gp1b = bcast(19, "b0")
            gpg = bcast(20, "b2")
            gpb = bcast(21, "b3")
            gp2b = bcast(22, "b1")

            hgT16 = [
                workp.tile([128, B], BF16, tag=f"hgT16_{jt}", name=f"hgT16_{jt}")
                for jt in range(HT)
            ]
            for jt in range(HT):
                ptz = psT.tile([128, 512], F32, tag="trans", name="trans")
                nc.tensor.transpose(
                    ptz[:, :B], hgsum[:B, jt * 128:(jt + 1) * 128], identf[:B, :B]
                )
                nc.vector.tensor_copy(out=hgT16[jt][:], in_=ptz[:, :B])

            p1 = psB.tile([128, H], F32, tag="ps_main", name="ps_main")
            for jt in range(HT):
                nc.tensor.matmul(
                    p1[:B, :], hgT16[jt][:, :B], GP1T_sb[jt][:],
                    start=(jt == 0), stop=(jt == HT - 1),
                )
            z1 = workp.tile([128, H], F32, tag="ub", name="ub")
            nc.vector.tensor_tensor(
                out=z1[:B, :], in0=p1[:B, :], in1=gp1b[:B, :], op=ALU.add
            )
            zg = workp.tile([128, H], BF16, tag="zg", name="zg")
            nc.vector.memset(zg[:], 0.0)
            ln_tile(z1[:B, :], zg[:B, :], eps1, p=B, g=gpg, b=gpb, gelu=True)
            zgT = [
                workp.tile([128, B], BF16, tag=f"zgT{jt}", name=f"zgT{jt}")
                for jt in range(HT)
            ]
            for jt in range(HT):
                ptz = psT.tile([128, 512], BF16, tag="trans", name="trans")
                nc.tensor.transpose(
                    ptz[:, :128], zg[:, jt * 128:(jt + 1) * 128], ident[:]
                )
                nc.vector.tensor_copy(out=zgT[jt][:], in_=ptz[:, :B])
            p2 = psB.tile([128, H], F32, tag="ps_main", name="ps_main")
            for jt in range(HT):
                nc.tensor.matmul(
                    p2[:B, :], zgT[jt][:, :B], GP2T_sb[jt][:],
                    start=(jt == 0), stop=(jt == HT - 1),
                )
            zout = workp.tile([128, H], F32, tag="zout", name="zout")
            nc.vector.tensor_tensor(
                out=zout[:B, :], in0=p2[:B, :], in1=gp2b[:B, :], op=ALU.add
            )
            nc.sync.dma_start(out[:], zout[:B, :])

    _split_multi_waits(nc)
    return nc


# ---------------------------------------------------------------------------
# host side
# ---------------------------------------------------------------------------


def _prepare_inputs(inputs):
    x = _f32(inputs["x"])
    edge_index = np.asarray(inputs["edge_index"])
    edge_attr = _f32(inputs["edge_attr"])
    batch = np.asarray(inputs["batch"]).astype(np.int64)
    g = {
        k: _f32(v)
        for k, v in inputs.items()
        if k not in ("x", "edge_index", "edge_attr", "batch")
    }

    dst = edge_index[1].astype(np.int64)
    perm = np.argsort(dst, kind="stable")
    dst_s = dst[perm]
    ea_s = edge_attr[perm]
    deg = np.bincount(dst, minlength=N).astype(np.float32)

    bounds_ok = all(
        dst_s[t * 128 - 1] != dst_s[t * 128] for t in range(1, E // 128)
    )
    node_ok = all(
        (dst_s[c * EC:(c + 1) * EC] >= c * NCC).all()
        and (dst_s[c * EC:(c + 1) * EC] < (c + 1) * NCC).all()
        for c in range(NC)
    )
    fast = bool(bounds_ok and node_ok)

    def ablk_for(c):
        rows = dst_s[c * EC:(c + 1) * EC]
        if fast:
            outb = np.zeros((ET, 128, 128), np.float32)
            for et in range(ET):
                seg = rows[et * 128:(et + 1) * 128]
                outb[et] = 2.0 * (seg[:, None] == seg[None, :])
            return _bf(outb)
        outb = np.zeros((ET, E // 128, 128, 128), np.float32)
        for et in range(ET):
            seg = rows[et * 128:(et + 1) * 128]
            for jt in range(E // 128):
                seg2 = dst_s[jt * 128:(jt + 1) * 128]
                outb[et, jt] = 2.0 * (seg2[:, None] == seg[None, :])
        return _bf(outb)

    def mtb_for(c):
        vlo = c * NCC
        cols = vlo + np.arange(NCC)
        if fast:
            outb = np.zeros((ET, 128, NCC), np.float32)
            for et in range(ET):
                seg = dst_s[c * EC + et * 128:c * EC + (et + 1) * 128]
                outb[et] = seg[:, None] == cols[None, :]
            return _bf(outb)
        outb = np.zeros((E // 128, 128, NCC), np.float32)
        for eb in range(E // 128):
            seg = dst_s[eb * 128:(eb + 1) * 128]
            outb[eb] = seg[:, None] == cols[None, :]
        return _bf(outb)

    qkv_W, qkv_b = g["qkv_W"], g["qkv_b"]
    ag, ab_ = g["attn_ln_g"], g["attn_ln_b"]
    WQT = np.zeros((L, H, H), np.float32)
    WKT = np.zeros((L, H, H), np.float32)
    WVT = np.zeros((L, H, H), np.float32)
    WOTT = np.zeros((L, H, H), np.float32)
    WUPT = np.zeros((L, H, H), np.float32)
    BQ = np.zeros((L, H, 1), np.float32)
    BK = np.zeros((L, H, 1), np.float32)
    BCAST = np.zeros((23, 128, H), np.float32)
    sc = 1.0 / np.sqrt(HD)
    for t in range(L):
        Wq, Wk, Wv = qkv_W[t, :H], qkv_W[t, H:2 * H], qkv_W[t, 2 * H:]
        bq, bk, bv = qkv_b[t, :H], qkv_b[t, H:2 * H], qkv_b[t, 2 * H:]
        Wq_e = Wq * ag[t][None, :]
        Wk_e = Wk * ag[t][None, :]
        Wv_e = Wv * ag[t][None, :]
        bq_e = bq + Wq @ ab_[t]
        bk_e = bk + Wk @ ab_[t]
        bv_e = bv + Wv @ ab_[t]
        WQT[t] = (Wq_e * sc).T
        WKT[t] = Wk_e.T
        WVT[t] = Wv_e.T
        BQ[t, :, 0] = bq_e * sc
        BK[t, :, 0] = bk_e
        BCAST[1 + t, :, :] = bv_e[None, :]
        wo, bo = g["attn_out_W"][t], g["attn_out_b"][t]
        WOTT[t] = wo.T
        updW, updb = g["upd_W"][t], g["upd_b"][t]
        WUPT[t] = updW.T
        BCAST[4 + t, :, :] = (updb + updW @ bo)[None, :]
        BCAST[7 + t, :, :] = g["upd_ln_g"][t][None, :]
        BCAST[10 + t, :, :] = g["upd_ln_b"][t][None, :]
    BCAST[0, :, :] = g["Wh_b"][None, :]
    BCAST[13, :, :] = g["atom_emb_b"][None, :]
    BCAST[14, :, :] = g["atom_ln_g"][None, :]
    BCAST[15, :, :] = g["atom_ln_b"][None, :]
    BCAST[16, :, :] = 2.0 * g["feat_b"][None, :]
    aqkv_W, aqkv_b = g["a_qkv_W"], g["a_qkv_b"]
    alg, alb = g["a_ln_g"], g["a_ln_b"]
    AWq, AWk, AWv = aqkv_W[:H], aqkv_W[H:2 * H], aqkv_W[2 * H:]
    Abq, Abk, Abv = aqkv_b[:H], aqkv_b[H:2 * H], aqkv_b[2 * H:]
    AWq_e = AWq * alg[None, :]
    AWk_e = AWk * alg[None, :]
    AWv_e = AWv * alg[None, :]
    BCAST[17, :, :] = (Abv + AWv @ alb)[None, :]
    BCAST[18, :, :] = g["a_out_b"][None, :]
    BCAST[19, :, :] = g["gp1_b"][None, :]
    BCAST[20, :, :] = g["gp_ln_g"][None, :]
    BCAST[21, :, :] = g["gp_ln_b"][None, :]
    BCAST[22, :, :] = g["gp2_b"][None, :]

    cnt = np.bincount(batch, minlength=B).astype(np.float32)
    cnt[cnt == 0] = 1.0

    shared = dict(
        WQT=_bf(WQT), WKT=_bf(WKT), WVT=_bf(WVT), WOTT=_bf(WOTT),
        WUPT=_bf(WUPT), BQ=_f32(BQ), BK=_f32(BK), BCAST=_bf(BCAST),
        bondWT=_bf(g["bond_emb_W"].T), bondB=_f32(g["bond_emb_b"][:, None]),
        WHT=_bf(g["Wh_W"].T),
        atomWT=_bf(g["atom_emb_W"].T),
        W1T=_bf(2.0 * g["feat_W"][:, :H].T),
        W2T=_bf(2.0 * g["feat_W"][:, H:].T),
        AQT=_bf((AWq_e * sc).T), AKT=_bf(AWk_e.T), AVT=_bf(AWv_e.T),
        BAQ=_f32(((Abq + AWq @ alb) * sc)[:, None]),
        BAK=_f32((Abk + AWk @ alb)[:, None]),
        AOTT=_bf(g["a_out_W"].T),
        GP1T=_bf(g["gp1_W"].T), GP2T=_bf(g["gp2_W"].T),
    )

    in_maps = []
    for c in range(NC):
        m = dict(shared)
        m["eaT"] = _bf(ea_s[c * EC:(c + 1) * EC].T)
        m["xT"] = _bf(x[c * NCC:(c + 1) * NCC].T)
        m["dege2"] = _f32(2.0 * deg[dst_s[c * EC:(c + 1) * EC]][:, None])
        m["cntinv"] = _f32((1.0 / cnt[batch[c * NCC:(c + 1) * NCC]])[:, None])
        pb = np.zeros((NCC, B), np.float32)
        pb[np.arange(NCC), batch[c * NCC:(c + 1) * NCC]] = 1.0
        m["PB"] = _bf(pb)
        m["Ablk"] = ablk_for(c)
        m["MtB"] = mtb_for(c)
        in_maps.append(m)
    return in_maps, fast


_CACHE = {}


def kernel(**inputs) -> np.ndarray:
    in_maps, fast = _prepare_inputs(inputs)
    if fast not in _CACHE:
        _CACHE[fast] = build_nc(fast)
    res = run_bass_kernel_spmd(_CACHE[fast], in_maps, list(range(NC)))
    return np.asarray(res.results[0]["out"], np.float32)
